# revision 11
# baseline (speedup 1.0000x reference)
"""Multi-head causal attention on 8 Trainium2 NeuronCores.

Sharding: core c -> (batch b = c//2, head-half hh = c%2).  Each core computes
q/k/v projections for its 8 heads (column-sharded wq/wk/wv), causal attention,
and a full-width partial output projection (row-sharded wo).  Host sums the
two partials per batch and adds the bias.

Device-side layout trick: scores are computed transposed (scoresT[j, i]) so
that the softmax-weighted sum over keys (ctx) is a plain matmul with v as the
stationary operand.  Ones-columns baked alongside v produce the softmax
denominator replicated across 64 partitions in the same PSUM tile as ctx.
"""

import numpy as np

import concourse.bass as bass
import concourse.mybir as mybir
import concourse.tile as tile
from concourse import bacc
from concourse.bass_utils import run_bass_kernel_spmd

# Problem shape (hardcoded; kernel.py must be self-contained).
B, S, D, H = 4, 2048, 1024, 16
HD = D // H           # 64 head dim
NCORES = 8
EH = D // 2           # 512: per-core e-width (8 heads)
NHL = H // 2          # 8 local heads per core
SB = 512              # s-block (free dim of most matmuls)
NSB = S // SB         # 4
NST = S // 128        # 16 s-tiles / j-tiles
NEG = EH // 128       # 4 e-groups of 128 partitions
NKG = D // 128        # 8 d-groups (contraction tiles)
VROW = 4 * 192        # v_ext row: 4x [v_even(64) | ones(64) | v_odd(64)] = 768

F32 = mybir.dt.float32
F32R = mybir.dt.float32r

TRACE = False
LAST_RESULT = None


def _build():
    nc = bacc.Bacc()

    xT_d = nc.dram_tensor("xt", [D, S], F32R, kind="ExternalInput")
    wqT_d = nc.dram_tensor("wqt", [D, EH], F32R, kind="ExternalInput")
    wkT_d = nc.dram_tensor("wkt", [D, EH], F32R, kind="ExternalInput")
    wvT_d = nc.dram_tensor("wvt", [D, EH], F32R, kind="ExternalInput")
    woT_d = nc.dram_tensor("wot", [EH, D], F32R, kind="ExternalInput")
    masks_d = nc.dram_tensor("masks", [4, 128, SB], F32R, kind="ExternalInput")
    out_d = nc.dram_tensor("out", [S, D], F32, kind="ExternalOutput")

    with tile.TileContext(nc) as tc:
        with (
            tc.tile_pool(name="persist", bufs=1) as persist,
            tc.tile_pool(name="accp", bufs=3, space="PSUM") as accp,
        ):
            ones_t = persist.tile([128, 128], F32R)
            nc.vector.memset(ones_t[:].bitcast(F32), 1.0)
            qT = persist.tile([128, NEG, S], F32R)      # [e-part, e-group, s]
            kT = persist.tile([128, NEG, S], F32R)
            v_ext = persist.tile([128, NST, VROW], F32R)  # [s-part, s-tile, row]

            # shared ones block between each (even, odd) head pair
            for st in range(NST):
                for p in range(4):
                    nc.vector.memset(
                        v_ext[:, st, p * 192 + 64 : p * 192 + 128].bitcast(F32),
                        1.0,
                    )

            # ---------------- Phase 1: projections ----------------
            with (
                tc.tile_pool(name="p1w", bufs=1) as p1w,
                tc.tile_pool(name="p1x", bufs=1) as p1x,
            ):
                w_q = p1w.tile([128, NKG, EH], F32R)
                w_k = p1w.tile([128, NKG, EH], F32R)
                w_v = p1w.tile([128, NKG, EH], F32R)
                for kg in range(NKG):
                    sl = slice(kg * 128, (kg + 1) * 128)
                    nc.sync.dma_start(out=w_q[:, kg, :], in_=wqT_d[sl, :])
                    nc.sync.dma_start(out=w_k[:, kg, :], in_=wkT_d[sl, :])
                    nc.sync.dma_start(out=w_v[:, kg, :], in_=wvT_d[sl, :])

                for sb in range(NSB):
                    ssl = slice(sb * SB, (sb + 1) * SB)
                    xts = p1x.tile([128, NKG, SB], F32R, tag="xts")
                    for kg in range(NKG):
                        nc.sync.dma_start(
                            out=xts[:, kg, :],
                            in_=xT_d[kg * 128 : (kg + 1) * 128, ssl],
                        )
                    # qT / kT blocks: out [e-part(128 of group mt), s(512)]
                    for w_sb, dst in ((w_q, qT), (w_k, kT)):
                        for mt in range(NEG):
                            ps = accp.tile([128, SB], F32, tag="acc")
                            msl = slice(mt * 128, (mt + 1) * 128)
                            for kg in range(NKG):
                                nc.tensor.matmul(
                                    out=ps,
                                    lhsT=(w_sb[:, kg, msl]),
                                    rhs=(xts[:, kg, :]),
                                    start=(kg == 0),
                                    stop=(kg == NKG - 1),
                                )
                            nc.vector.tensor_copy(dst[:, mt, ssl], ps)
                    # v blocks: out [s-part(128 of tile st), e(512)]
                    for st4 in range(SB // 128):
                        st = sb * (SB // 128) + st4
                        ps = accp.tile([128, EH], F32, tag="acc")
                        xsl = slice(st4 * 128, (st4 + 1) * 128)
                        for kg in range(NKG):
                            nc.tensor.matmul(
                                out=ps,
                                lhsT=(xts[:, kg, xsl]),
                                rhs=(w_v[:, kg, :]),
                                start=(kg == 0),
                                stop=(kg == NKG - 1),
                            )
                        # psum cols: head h at [h*64, h*64+64); dest pair p:
                        # even head -> p*192, odd head -> p*192+128
                        psr = ps[:].rearrange("p (a c) -> p a c", c=128)
                        vst = v_ext[:, st, :].rearrange("p (a w) -> p a w", w=192)
                        nc.vector.tensor_copy(vst[:, :, 0:64], psr[:, :, 0:64])
                        nc.vector.tensor_copy(vst[:, :, 128:192], psr[:, :, 64:128])

            # ---------------- Phase 2: attention ----------------
            with (
                tc.tile_pool(name="p2c", bufs=1) as p2c,
                tc.tile_pool(name="ctxp", bufs=1) as ctxp,
                tc.tile_pool(name="expp", bufs=4) as expp,
                tc.tile_pool(name="sp", bufs=3, space="PSUM") as sp,
                tc.tile_pool(name="bcp", bufs=2, space="PSUM") as bcp,
                tc.tile_pool(name="smallp", bufs=2) as smallp,
            ):
                masks_sb = p2c.tile([128, 4, SB], F32R)
                for r in range(4):
                    nc.sync.dma_start(out=masks_sb[:, r, :], in_=masks_d[r])

                ctxT = ctxp.tile([128, NEG, S], F32R)
                for h in range(NHL):
                    po = (h % 2) * 64          # partition offset of this head
                    g = h // 2                 # e-group of this head
                    hsl = slice(po, po + 64)
                    for ib in range(NSB):
                        isl = slice(ib * SB, (ib + 1) * SB)
                        njt = 4 * (ib + 1)
                        ps_ctx = accp.tile([128, SB], F32, tag="acc")
                        for jt in range(njt):
                            ps_s = sp.tile([128, SB], F32, tag="s")
                            nc.tensor.matmul(
                                out=ps_s,
                                lhsT=(kT[hsl, g, jt * 128 : (jt + 1) * 128]),
                                rhs=(qT[hsl, g, isl]),
                                start=True,
                                stop=True,
                            )
                            expT = expp.tile([128, SB], F32R, tag="exp")
                            nc.scalar.activation(
                                out=expT,
                                in_=ps_s,
                                func=mybir.ActivationFunctionType.Exp,
                                scale=1.0 / np.sqrt(HD),
                            )
                            r = jt - 4 * ib
                            if r >= 0:
                                w = 128 * (r + 1)
                                nc.vector.tensor_mul(
                                    expT[:, 0:w], expT[:, 0:w], masks_sb[:, r, 0:w]
                                )
                            # stationary operand: [v_h | ones] (even h) or
                            # [ones | v_h] (odd h) -- contiguous 128 cols
                            coff = (h // 2) * 192 + (0 if h % 2 == 0 else 64)
                            lhsT_v = v_ext[:, jt, coff : coff + 128]
                            nc.tensor.matmul(
                                out=ps_ctx,
                                lhsT=(lhsT_v),
                                rhs=(expT),
                                start=(jt == 0),
                                stop=(jt == njt - 1),
                            )
                        # rows [po:po+64] = ctx, rows [64-po:128-po] = denom
                        do = 64 - po
                        rdt = smallp.tile([128, SB], F32R, tag="rdt")
                        with nc.allow_low_precision(
                            "f32r rounding of softmax denominators"
                        ):
                            nc.vector.reciprocal(
                                rdt[do : do + 1, :], ps_ctx[do : do + 1, :]
                            )
                        # broadcast recip row to all 128 partitions: PE outer
                        # product ones[1,128].T @ rdt_row[1,512]
                        bc_ps = bcp.tile([128, SB], F32, tag="bc")
                        nc.tensor.matmul(
                            out=bc_ps,
                            lhsT=ones_t[do : do + 1, :],
                            rhs=rdt[do : do + 1, :],
                            start=True,
                            stop=True,
                        )
                        bc = smallp.tile([128, SB], F32, tag="bc")
                        nc.vector.tensor_copy(bc[hsl, :], bc_ps[hsl, :])
                        nc.vector.tensor_mul(
                            ctxT[hsl, g, isl], ps_ctx[hsl, :], bc[hsl, :]
                        )

                # ---------------- Phase 3: output projection ----------------
                with tc.tile_pool(name="p3", bufs=2) as p3:
                    woT_sb = p2c.tile([128, NEG, D], F32R)
                    for gg in range(NEG):
                        nc.sync.dma_start(
                            out=woT_sb[:, gg, :],
                            in_=woT_d[gg * 128 : (gg + 1) * 128, :],
                        )
                    for it in range(NST):
                        itsl = slice(it * 128, (it + 1) * 128)
                        for ob in range(2):
                            osl = slice(ob * SB, (ob + 1) * SB)
                            ps = accp.tile([128, SB], F32, tag="acc")
                            for gg in range(NEG):
                                nc.tensor.matmul(
                                    out=ps,
                                    lhsT=(ctxT[:, gg, itsl]),
                                    rhs=(woT_sb[:, gg, osl]),
                                    start=(gg == 0),
                                    stop=(gg == NEG - 1),
                                )
                            ot = p3.tile([128, SB], F32, tag="ot")
                            nc.vector.tensor_copy(ot, ps)
                            nc.sync.dma_start(out=out_d[itsl, osl], in_=ot)

    nc.finalize()
    return nc


_NC = None


def _get_nc():
    global _NC
    if _NC is None:
        _NC = _build()
    return _NC


def kernel(x, wq, wk, wv, wo, wo_b):
    global LAST_RESULT
    x = np.ascontiguousarray(np.asarray(x, dtype=np.float32))
    wq = np.asarray(wq, dtype=np.float32)
    wk = np.asarray(wk, dtype=np.float32)
    wv = np.asarray(wv, dtype=np.float32)
    wo = np.asarray(wo, dtype=np.float32)
    wo_b = np.asarray(wo_b, dtype=np.float32)

    rr, pp, ff = np.ogrid[0:4, 0:128, 0:SB]
    masks = (128 * rr + pp <= ff).astype(np.float32)

    in_maps = []
    for c in range(NCORES):
        b, hh = c // 2, c % 2
        es = slice(hh * EH, (hh + 1) * EH)
        in_maps.append(
            {
                "xt": np.ascontiguousarray(x[b].T),
                "wqt": np.ascontiguousarray(wq[es, :].T),
                "wkt": np.ascontiguousarray(wk[es, :].T),
                "wvt": np.ascontiguousarray(wv[es, :].T),
                "wot": np.ascontiguousarray(wo[:, es].T),
                "masks": masks,
            }
        )

    nc = _get_nc()
    res = run_bass_kernel_spmd(nc, in_maps, list(range(NCORES)), trace=TRACE)
    LAST_RESULT = res

    out = np.empty((B, S, D), np.float32)
    for b in range(B):
        out[b] = res.results[2 * b]["out"] + res.results[2 * b + 1]["out"]
    out += wo_b[None, None, :]
    return out


# revision 14
# speedup vs baseline: 1.1025x; 1.1025x over previous
"""Multi-head causal attention on 8 Trainium2 NeuronCores.

Sharding: core c -> (batch b = c//2, head-half hh = c%2).  Each core computes
q/k/v projections for its 8 heads (column-sharded wq/wk/wv), causal attention,
and a full-width partial output projection (row-sharded wo).  Host sums the
two partials per batch and adds the bias.

Device-side layout trick: scores are computed transposed (scoresT[j, i]) so
that the softmax-weighted sum over keys (ctx) is a plain matmul with v as the
stationary operand.  Ones-columns baked alongside v produce the softmax
denominator replicated across 64 partitions in the same PSUM tile as ctx.
"""

import numpy as np

import concourse.bass as bass
import concourse.mybir as mybir
import concourse.tile as tile
from concourse import bacc
from concourse.bass_utils import run_bass_kernel_spmd

# Problem shape (hardcoded; kernel.py must be self-contained).
B, S, D, H = 4, 2048, 1024, 16
HD = D // H           # 64 head dim
NCORES = 8
EH = D // 2           # 512: per-core e-width (8 heads)
NHL = H // 2          # 8 local heads per core
SB = 512              # s-block (free dim of most matmuls)
NSB = S // SB         # 4
NST = S // 128        # 16 s-tiles / j-tiles
NEG = EH // 128       # 4 e-groups of 128 partitions
NKG = D // 128        # 8 d-groups (contraction tiles)
VROW = 4 * 192        # v_ext row: 4x [v_even(64) | ones(64) | v_odd(64)] = 768

F32 = mybir.dt.float32
F32R = mybir.dt.float32r

TRACE = False
LAST_RESULT = None


def _build():
    nc = bacc.Bacc()

    xT_d = nc.dram_tensor("xt", [D, S], F32R, kind="ExternalInput")
    wqT_d = nc.dram_tensor("wqt", [D, EH], F32R, kind="ExternalInput")
    wkT_d = nc.dram_tensor("wkt", [D, EH], F32R, kind="ExternalInput")
    wvT_d = nc.dram_tensor("wvt", [D, EH], F32R, kind="ExternalInput")
    woT_d = nc.dram_tensor("wot", [EH, D], F32R, kind="ExternalInput")
    masks_d = nc.dram_tensor("masks", [128, 128], F32R, kind="ExternalInput")
    out_d = nc.dram_tensor("out", [S, D], F32, kind="ExternalOutput")

    with tile.TileContext(nc) as tc:
        with (
            tc.tile_pool(name="persist", bufs=1) as persist,
            tc.tile_pool(name="accp", bufs=3, space="PSUM") as accp,
        ):
            ones_t = persist.tile([128, 128], F32)
            nc.vector.memset(ones_t, 1.0)
            qT = persist.tile([128, NEG, S], F32R)      # [e-part, e-group, s]
            kT = persist.tile([128, NEG, S], F32R)
            v_ext = persist.tile([128, NST, VROW], F32R)  # [s-part, s-tile, row]

            # shared ones block between each (even, odd) head pair
            for st in range(NST):
                for p in range(4):
                    nc.vector.memset(
                        v_ext[:, st, p * 192 + 64 : p * 192 + 128].bitcast(F32),
                        1.0,
                    )

            # ---------------- Phase 1: projections ----------------
            with (
                tc.tile_pool(name="p1w", bufs=1) as p1w,
                tc.tile_pool(name="p1x", bufs=2) as p1x,
            ):
                w_q = p1w.tile([128, NKG, EH], F32R)
                w_k = p1w.tile([128, NKG, EH], F32R)
                w_v = p1w.tile([128, NKG, EH], F32R)
                for kg in range(NKG):
                    sl = slice(kg * 128, (kg + 1) * 128)
                    nc.sync.dma_start(out=w_q[:, kg, :], in_=wqT_d[sl, :])
                    nc.sync.dma_start(out=w_k[:, kg, :], in_=wkT_d[sl, :])
                    nc.sync.dma_start(out=w_v[:, kg, :], in_=wvT_d[sl, :])

                for sb in range(NSB):
                    ssl = slice(sb * SB, (sb + 1) * SB)
                    xts = p1x.tile([128, NKG, SB], F32R, tag="xts")
                    for kg in range(NKG):
                        nc.sync.dma_start(
                            out=xts[:, kg, :],
                            in_=xT_d[kg * 128 : (kg + 1) * 128, ssl],
                        )
                    # qT / kT blocks: out [e-part(128 of group mt), s(512)]
                    for w_sb, dst in ((w_q, qT), (w_k, kT)):
                        for mt in range(NEG):
                            ps = accp.tile([128, SB], F32, tag="acc")
                            msl = slice(mt * 128, (mt + 1) * 128)
                            for kg in range(NKG):
                                nc.tensor.matmul(
                                    out=ps,
                                    lhsT=(w_sb[:, kg, msl]),
                                    rhs=(xts[:, kg, :]),
                                    start=(kg == 0),
                                    stop=(kg == NKG - 1),
                                )
                            nc.vector.tensor_copy(dst[:, mt, ssl], ps)
                    # v blocks: out [s-part(128 of tile st), e(512)]
                    for st4 in range(SB // 128):
                        st = sb * (SB // 128) + st4
                        ps = accp.tile([128, EH], F32, tag="acc")
                        xsl = slice(st4 * 128, (st4 + 1) * 128)
                        for kg in range(NKG):
                            nc.tensor.matmul(
                                out=ps,
                                lhsT=(xts[:, kg, xsl]),
                                rhs=(w_v[:, kg, :]),
                                start=(kg == 0),
                                stop=(kg == NKG - 1),
                            )
                        # psum cols: head h at [h*64, h*64+64); dest pair p:
                        # even head -> p*192, odd head -> p*192+128
                        psr = ps[:].rearrange("p (a c) -> p a c", c=128)
                        vst = v_ext[:, st, :].rearrange("p (a w) -> p a w", w=192)
                        nc.vector.tensor_copy(vst[:, :, 0:64], psr[:, :, 0:64])
                        nc.vector.tensor_copy(vst[:, :, 128:192], psr[:, :, 64:128])

            # ---------------- Phase 2: attention ----------------
            with (
                tc.tile_pool(name="p2c", bufs=1) as p2c,
                tc.tile_pool(name="ctxp", bufs=1) as ctxp,
                tc.tile_pool(name="expp", bufs=4) as expp,
                tc.tile_pool(name="sp", bufs=4, space="PSUM") as sp,
                tc.tile_pool(name="bcp", bufs=1, space="PSUM") as bcp,
                tc.tile_pool(name="smallp", bufs=2) as smallp,
            ):
                masks_sb = p2c.tile([128, 128], F32R)
                nc.sync.dma_start(out=masks_sb, in_=masks_d[:, :])

                ctxT = ctxp.tile([128, NEG, S], F32R)
                for h in range(NHL):
                    po = (h % 2) * 64          # partition offset of this head
                    g = h // 2                 # e-group of this head
                    hsl = slice(po, po + 64)
                    for ib in range(NSB):
                        isl = slice(ib * SB, (ib + 1) * SB)
                        njt = 4 * (ib + 1)
                        ps_ctx = accp.tile([128, SB], F32, tag="acc")
                        for jt in range(njt):
                            r = jt - 4 * ib
                            f0 = 128 * r if r > 0 else 0   # causal: cols < f0 are all-masked
                            ps_s = sp.tile([128, SB], F32, tag="s")
                            nc.tensor.matmul(
                                out=ps_s[:, f0:SB],
                                lhsT=(kT[hsl, g, jt * 128 : (jt + 1) * 128]),
                                rhs=(qT[hsl, g, ib * SB + f0 : (ib + 1) * SB]),
                                start=True,
                                stop=True,
                            )
                            expT = expp.tile([128, SB], F32R, tag="exp")
                            nc.scalar.activation(
                                out=expT[:, f0:SB],
                                in_=ps_s[:, f0:SB],
                                func=mybir.ActivationFunctionType.Exp,
                                scale=1.0 / np.sqrt(HD),
                            )
                            if r >= 0:
                                # triangular mask on the 128-wide diagonal block
                                nc.vector.tensor_mul(
                                    expT[:, f0 : f0 + 128],
                                    expT[:, f0 : f0 + 128],
                                    masks_sb,
                                )
                            # stationary operand: [v_h | ones] (even h) or
                            # [ones | v_h] (odd h) -- contiguous 128 cols
                            coff = (h // 2) * 192 + (0 if h % 2 == 0 else 64)
                            lhsT_v = v_ext[:, jt, coff : coff + 128]
                            nc.tensor.matmul(
                                out=ps_ctx[:, f0:SB],
                                lhsT=(lhsT_v),
                                rhs=(expT[:, f0:SB]),
                                start=(jt == 0),
                                stop=(jt == njt - 1),
                            )
                        # rows [po:po+64] = ctx, rows [64-po:128-po] = denom
                        do = 64 - po
                        rdt = smallp.tile([128, SB], F32, tag="rdt")
                        nc.vector.reciprocal(
                            rdt[do : do + 1, :], ps_ctx[do : do + 1, :]
                        )
                        # broadcast recip row to all 128 partitions: PE outer
                        # product ones[1,128].T @ rdt_row[1,512]
                        bc_ps = bcp.tile([128, SB], F32, tag="bc")
                        nc.tensor.matmul(
                            out=bc_ps,
                            lhsT=ones_t[do : do + 1, :],
                            rhs=rdt[do : do + 1, :],
                            start=True,
                            stop=True,
                        )
                        bc = smallp.tile([128, SB], F32, tag="bc")
                        nc.vector.tensor_copy(bc[hsl, :], bc_ps[hsl, :])
                        nc.vector.tensor_mul(
                            ctxT[hsl, g, isl], ps_ctx[hsl, :], bc[hsl, :]
                        )

                # ---------------- Phase 3: output projection ----------------
                with tc.tile_pool(name="p3", bufs=2) as p3:
                    woT_sb = p2c.tile([128, NEG, D], F32R)
                    for gg in range(NEG):
                        nc.sync.dma_start(
                            out=woT_sb[:, gg, :],
                            in_=woT_d[gg * 128 : (gg + 1) * 128, :],
                        )
                    for it in range(NST):
                        itsl = slice(it * 128, (it + 1) * 128)
                        for ob in range(2):
                            osl = slice(ob * SB, (ob + 1) * SB)
                            ps = accp.tile([128, SB], F32, tag="acc")
                            for gg in range(NEG):
                                nc.tensor.matmul(
                                    out=ps,
                                    lhsT=(ctxT[:, gg, itsl]),
                                    rhs=(woT_sb[:, gg, osl]),
                                    start=(gg == 0),
                                    stop=(gg == NEG - 1),
                                )
                            ot = p3.tile([128, SB], F32, tag="ot")
                            nc.vector.tensor_copy(ot, ps)
                            nc.sync.dma_start(out=out_d[itsl, osl], in_=ot)

    nc.finalize()
    return nc


_NC = None


def _get_nc():
    global _NC
    if _NC is None:
        _NC = _build()
    return _NC


def kernel(x, wq, wk, wv, wo, wo_b):
    global LAST_RESULT
    x = np.ascontiguousarray(np.asarray(x, dtype=np.float32))
    wq = np.asarray(wq, dtype=np.float32)
    wk = np.asarray(wk, dtype=np.float32)
    wv = np.asarray(wv, dtype=np.float32)
    wo = np.asarray(wo, dtype=np.float32)
    wo_b = np.asarray(wo_b, dtype=np.float32)

    pp, ff = np.ogrid[0:128, 0:128]
    masks = (pp <= ff).astype(np.float32)

    in_maps = []
    for c in range(NCORES):
        b, hh = c // 2, c % 2
        es = slice(hh * EH, (hh + 1) * EH)
        in_maps.append(
            {
                "xt": np.ascontiguousarray(x[b].T),
                "wqt": np.ascontiguousarray(wq[es, :].T),
                "wkt": np.ascontiguousarray(wk[es, :].T),
                "wvt": np.ascontiguousarray(wv[es, :].T),
                "wot": np.ascontiguousarray(wo[:, es].T),
                "masks": masks,
            }
        )

    nc = _get_nc()
    res = run_bass_kernel_spmd(nc, in_maps, list(range(NCORES)), trace=TRACE)
    LAST_RESULT = res

    out = np.empty((B, S, D), np.float32)
    for b in range(B):
        out[b] = res.results[2 * b]["out"] + res.results[2 * b + 1]["out"]
    out += wo_b[None, None, :]
    return out


# revision 15
# speedup vs baseline: 1.1945x; 1.0834x over previous
"""Multi-head causal attention on 8 Trainium2 NeuronCores.

Sharding: core c -> (batch b = c//2, head-half hh = c%2).  Each core computes
q/k/v projections for its 8 heads (column-sharded wq/wk/wv), causal attention,
and a full-width partial output projection (row-sharded wo).  Host sums the
two partials per batch and adds the bias.

Device-side layout trick: scores are computed transposed (scoresT[j, i]) so
that the softmax-weighted sum over keys (ctx) is a plain matmul with v as the
stationary operand.  Ones-columns baked alongside v produce the softmax
denominator replicated across 64 partitions in the same PSUM tile as ctx.
"""

import numpy as np

import concourse.bass as bass
import concourse.mybir as mybir
import concourse.tile as tile
from concourse import bacc
from concourse.bass_utils import run_bass_kernel_spmd

# Problem shape (hardcoded; kernel.py must be self-contained).
B, S, D, H = 4, 2048, 1024, 16
HD = D // H           # 64 head dim
NCORES = 8
EH = D // 2           # 512: per-core e-width (8 heads)
NHL = H // 2          # 8 local heads per core
SB = 512              # s-block (free dim of most matmuls)
NSB = S // SB         # 4
NST = S // 128        # 16 s-tiles / j-tiles
NEG = EH // 128       # 4 e-groups of 128 partitions
NKG = D // 128        # 8 d-groups (contraction tiles)
VROW = 4 * 192        # v_ext row: 4x [v_even(64) | ones(64) | v_odd(64)] = 768

F32 = mybir.dt.float32
F32R = mybir.dt.float32r
BF16 = mybir.dt.bfloat16
MMDT = BF16          # dtype for matmul inputs (BF16 or F32R)
import ml_dtypes
MMNP = ml_dtypes.bfloat16 if MMDT == BF16 else np.float32

TRACE = False
LAST_RESULT = None


def _build():
    nc = bacc.Bacc()

    xT_d = nc.dram_tensor("xt", [D, S], MMDT, kind="ExternalInput")
    wqT_d = nc.dram_tensor("wqt", [D, EH], MMDT, kind="ExternalInput")
    wkT_d = nc.dram_tensor("wkt", [D, EH], MMDT, kind="ExternalInput")
    wvT_d = nc.dram_tensor("wvt", [D, EH], MMDT, kind="ExternalInput")
    woT_d = nc.dram_tensor("wot", [EH, D], MMDT, kind="ExternalInput")
    masks_d = nc.dram_tensor("masks", [128, 128], MMDT, kind="ExternalInput")
    out_d = nc.dram_tensor("out", [S, D], F32, kind="ExternalOutput")

    with tile.TileContext(nc) as tc:
        with (
            tc.tile_pool(name="persist", bufs=1) as persist,
            tc.tile_pool(name="accp", bufs=3, space="PSUM") as accp,
        ):
            ones_t = persist.tile([128, 128], F32)
            nc.vector.memset(ones_t, 1.0)
            qT = persist.tile([128, NEG, S], MMDT)      # [e-part, e-group, s]
            kT = persist.tile([128, NEG, S], MMDT)
            v_ext = persist.tile([128, NST, VROW], MMDT)  # [s-part, s-tile, row]

            # shared ones block between each (even, odd) head pair
            for st in range(NST):
                for p in range(4):
                    ones_ap = v_ext[:, st, p * 192 + 64 : p * 192 + 128]
                    if MMDT == F32R:
                        ones_ap = ones_ap.bitcast(F32)
                    nc.vector.memset(ones_ap, 1.0)

            # ---------------- Phase 1: projections ----------------
            with (
                tc.tile_pool(name="p1w", bufs=1) as p1w,
                tc.tile_pool(name="p1x", bufs=2) as p1x,
            ):
                w_q = p1w.tile([128, NKG, EH], MMDT)
                w_k = p1w.tile([128, NKG, EH], MMDT)
                w_v = p1w.tile([128, NKG, EH], MMDT)
                for kg in range(NKG):
                    sl = slice(kg * 128, (kg + 1) * 128)
                    nc.sync.dma_start(out=w_q[:, kg, :], in_=wqT_d[sl, :])
                    nc.sync.dma_start(out=w_k[:, kg, :], in_=wkT_d[sl, :])
                    nc.sync.dma_start(out=w_v[:, kg, :], in_=wvT_d[sl, :])

                for sb in range(NSB):
                    ssl = slice(sb * SB, (sb + 1) * SB)
                    xts = p1x.tile([128, NKG, SB], MMDT, tag="xts")
                    for kg in range(NKG):
                        nc.sync.dma_start(
                            out=xts[:, kg, :],
                            in_=xT_d[kg * 128 : (kg + 1) * 128, ssl],
                        )
                    # qT / kT blocks: out [e-part(128 of group mt), s(512)]
                    for w_sb, dst in ((w_q, qT), (w_k, kT)):
                        for mt in range(NEG):
                            ps = accp.tile([128, SB], F32, tag="acc")
                            msl = slice(mt * 128, (mt + 1) * 128)
                            for kg in range(NKG):
                                nc.tensor.matmul(
                                    out=ps,
                                    lhsT=(w_sb[:, kg, msl]),
                                    rhs=(xts[:, kg, :]),
                                    start=(kg == 0),
                                    stop=(kg == NKG - 1),
                                )
                            nc.vector.tensor_copy(dst[:, mt, ssl], ps)
                    # v blocks: out [s-part(128 of tile st), e(512)]
                    for st4 in range(SB // 128):
                        st = sb * (SB // 128) + st4
                        ps = accp.tile([128, EH], F32, tag="acc")
                        xsl = slice(st4 * 128, (st4 + 1) * 128)
                        for kg in range(NKG):
                            nc.tensor.matmul(
                                out=ps,
                                lhsT=(xts[:, kg, xsl]),
                                rhs=(w_v[:, kg, :]),
                                start=(kg == 0),
                                stop=(kg == NKG - 1),
                            )
                        # psum cols: head h at [h*64, h*64+64); dest pair p:
                        # even head -> p*192, odd head -> p*192+128
                        psr = ps[:].rearrange("p (a c) -> p a c", c=128)
                        vst = v_ext[:, st, :].rearrange("p (a w) -> p a w", w=192)
                        nc.vector.tensor_copy(vst[:, :, 0:64], psr[:, :, 0:64])
                        nc.vector.tensor_copy(vst[:, :, 128:192], psr[:, :, 64:128])

            # ---------------- Phase 2: attention ----------------
            with (
                tc.tile_pool(name="p2c", bufs=1) as p2c,
                tc.tile_pool(name="ctxp", bufs=1) as ctxp,
                tc.tile_pool(name="expp", bufs=4) as expp,
                tc.tile_pool(name="sp", bufs=4, space="PSUM") as sp,
                tc.tile_pool(name="bcp", bufs=1, space="PSUM") as bcp,
                tc.tile_pool(name="smallp", bufs=2) as smallp,
            ):
                masks_sb = p2c.tile([128, 128], MMDT)
                nc.sync.dma_start(out=masks_sb, in_=masks_d[:, :])

                ctxT = ctxp.tile([128, NEG, S], MMDT)
                for h in range(NHL):
                    po = (h % 2) * 64          # partition offset of this head
                    g = h // 2                 # e-group of this head
                    hsl = slice(po, po + 64)
                    for ib in range(NSB):
                        isl = slice(ib * SB, (ib + 1) * SB)
                        njt = 4 * (ib + 1)
                        ps_ctx = accp.tile([128, SB], F32, tag="acc")
                        for jt in range(njt):
                            r = jt - 4 * ib
                            f0 = 128 * r if r > 0 else 0   # causal: cols < f0 are all-masked
                            ps_s = sp.tile([128, SB], F32, tag="s")
                            nc.tensor.matmul(
                                out=ps_s[:, f0:SB],
                                lhsT=(kT[hsl, g, jt * 128 : (jt + 1) * 128]),
                                rhs=(qT[hsl, g, ib * SB + f0 : (ib + 1) * SB]),
                                start=True,
                                stop=True,
                            )
                            expT = expp.tile([128, SB], MMDT, tag="exp")
                            nc.scalar.activation(
                                out=expT[:, f0:SB],
                                in_=ps_s[:, f0:SB],
                                func=mybir.ActivationFunctionType.Exp,
                                scale=1.0 / np.sqrt(HD),
                            )
                            if r >= 0:
                                # triangular mask on the 128-wide diagonal block
                                nc.vector.tensor_mul(
                                    expT[:, f0 : f0 + 128],
                                    expT[:, f0 : f0 + 128],
                                    masks_sb,
                                )
                            # stationary operand: [v_h | ones] (even h) or
                            # [ones | v_h] (odd h) -- contiguous 128 cols
                            coff = (h // 2) * 192 + (0 if h % 2 == 0 else 64)
                            lhsT_v = v_ext[:, jt, coff : coff + 128]
                            nc.tensor.matmul(
                                out=ps_ctx[:, f0:SB],
                                lhsT=(lhsT_v),
                                rhs=(expT[:, f0:SB]),
                                start=(jt == 0),
                                stop=(jt == njt - 1),
                            )
                        # rows [po:po+64] = ctx, rows [64-po:128-po] = denom
                        do = 64 - po
                        rdt = smallp.tile([128, SB], F32, tag="rdt")
                        nc.vector.reciprocal(
                            rdt[do : do + 1, :], ps_ctx[do : do + 1, :]
                        )
                        # broadcast recip row to all 128 partitions: PE outer
                        # product ones[1,128].T @ rdt_row[1,512]
                        bc_ps = bcp.tile([128, SB], F32, tag="bc")
                        nc.tensor.matmul(
                            out=bc_ps,
                            lhsT=ones_t[do : do + 1, :],
                            rhs=rdt[do : do + 1, :],
                            start=True,
                            stop=True,
                        )
                        bc = smallp.tile([128, SB], F32, tag="bc")
                        nc.vector.tensor_copy(bc[hsl, :], bc_ps[hsl, :])
                        nc.vector.tensor_mul(
                            ctxT[hsl, g, isl], ps_ctx[hsl, :], bc[hsl, :]
                        )

                # ---------------- Phase 3: output projection ----------------
                with tc.tile_pool(name="p3", bufs=2) as p3:
                    woT_sb = p2c.tile([128, NEG, D], MMDT)
                    for gg in range(NEG):
                        nc.sync.dma_start(
                            out=woT_sb[:, gg, :],
                            in_=woT_d[gg * 128 : (gg + 1) * 128, :],
                        )
                    for it in range(NST):
                        itsl = slice(it * 128, (it + 1) * 128)
                        for ob in range(2):
                            osl = slice(ob * SB, (ob + 1) * SB)
                            ps = accp.tile([128, SB], F32, tag="acc")
                            for gg in range(NEG):
                                nc.tensor.matmul(
                                    out=ps,
                                    lhsT=(ctxT[:, gg, itsl]),
                                    rhs=(woT_sb[:, gg, osl]),
                                    start=(gg == 0),
                                    stop=(gg == NEG - 1),
                                )
                            ot = p3.tile([128, SB], F32, tag="ot")
                            nc.vector.tensor_copy(ot, ps)
                            nc.sync.dma_start(out=out_d[itsl, osl], in_=ot)

    nc.finalize()
    return nc


_NC = None


def _get_nc():
    global _NC
    if _NC is None:
        _NC = _build()
    return _NC


def kernel(x, wq, wk, wv, wo, wo_b):
    global LAST_RESULT
    x = np.ascontiguousarray(np.asarray(x, dtype=np.float32))
    wq = np.asarray(wq, dtype=np.float32)
    wk = np.asarray(wk, dtype=np.float32)
    wv = np.asarray(wv, dtype=np.float32)
    wo = np.asarray(wo, dtype=np.float32)
    wo_b = np.asarray(wo_b, dtype=np.float32)

    pp, ff = np.ogrid[0:128, 0:128]
    masks = (pp <= ff).astype(np.float32)

    in_maps = []
    for c in range(NCORES):
        b, hh = c // 2, c % 2
        es = slice(hh * EH, (hh + 1) * EH)
        in_maps.append(
            {
                "xt": np.ascontiguousarray(x[b].T.astype(MMNP)),
                "wqt": np.ascontiguousarray(wq[es, :].T.astype(MMNP)),
                "wkt": np.ascontiguousarray(wk[es, :].T.astype(MMNP)),
                "wvt": np.ascontiguousarray(wv[es, :].T.astype(MMNP)),
                "wot": np.ascontiguousarray(wo[:, es].T.astype(MMNP)),
                "masks": masks.astype(MMNP),
            }
        )

    nc = _get_nc()
    res = run_bass_kernel_spmd(nc, in_maps, list(range(NCORES)), trace=TRACE)
    LAST_RESULT = res

    out = np.empty((B, S, D), np.float32)
    for b in range(B):
        out[b] = res.results[2 * b]["out"] + res.results[2 * b + 1]["out"]
    out += wo_b[None, None, :]
    return out


# revision 16
# speedup vs baseline: 1.3600x; 1.1386x over previous
"""Multi-head causal attention on 8 Trainium2 NeuronCores.

Sharding: core c -> (batch b = c//2, head-half hh = c%2).  Each core computes
q/k/v projections for its 8 heads (column-sharded wq/wk/wv), causal attention,
and a full-width partial output projection (row-sharded wo).  Host sums the
two partials per batch and adds the bias.

Device-side layout trick: scores are computed transposed (scoresT[j, i]) so
that the softmax-weighted sum over keys (ctx) is a plain matmul with v as the
stationary operand.  Ones-columns baked alongside v produce the softmax
denominator replicated across 64 partitions in the same PSUM tile as ctx.
"""

import numpy as np

import concourse.bass as bass
import concourse.mybir as mybir
import concourse.tile as tile
from concourse import bacc
from concourse.bass_utils import run_bass_kernel_spmd

# Problem shape (hardcoded; kernel.py must be self-contained).
B, S, D, H = 4, 2048, 1024, 16
HD = D // H           # 64 head dim
NCORES = 8
EH = D // 2           # 512: per-core e-width (8 heads)
NHL = H // 2          # 8 local heads per core
SB = 512              # s-block (free dim of most matmuls)
NSB = S // SB         # 4
NST = S // 128        # 16 s-tiles / j-tiles
NEG = EH // 128       # 4 e-groups of 128 partitions
NKG = D // 128        # 8 d-groups (contraction tiles)
VROW = 4 * 192        # v_ext row: 4x [v_even(64) | ones(64) | v_odd(64)] = 768

F32 = mybir.dt.float32
F32R = mybir.dt.float32r
BF16 = mybir.dt.bfloat16
MMDT = F32R          # dtype for matmul inputs (BF16 or F32R)
import ml_dtypes
MMNP = ml_dtypes.bfloat16 if MMDT == BF16 else np.float32

TRACE = False
LAST_RESULT = None


def _build():
    nc = bacc.Bacc()

    xT_d = nc.dram_tensor("xt", [D, S], MMDT, kind="ExternalInput")
    wqT_d = nc.dram_tensor("wqt", [D, EH], MMDT, kind="ExternalInput")
    wkT_d = nc.dram_tensor("wkt", [D, EH], MMDT, kind="ExternalInput")
    wvT_d = nc.dram_tensor("wvt", [D, EH], MMDT, kind="ExternalInput")
    woT_d = nc.dram_tensor("wot", [EH, D], MMDT, kind="ExternalInput")
    masks_d = nc.dram_tensor("masks", [128, 128], MMDT, kind="ExternalInput")
    out_d = nc.dram_tensor("out", [S, D], F32, kind="ExternalOutput")

    with tile.TileContext(nc) as tc:
        with (
            tc.tile_pool(name="persist", bufs=1) as persist,
            tc.tile_pool(name="accp", bufs=3, space="PSUM") as accp,
        ):
            ones_t = persist.tile([128, 128], F32)
            nc.vector.memset(ones_t, 1.0)
            qT = persist.tile([128, NEG, S], MMDT)      # [e-part, e-group, s]
            kT = persist.tile([128, NEG, S], MMDT)
            v_ext = persist.tile([128, NST, VROW], MMDT)  # [s-part, s-tile, row]

            # shared ones block between each (even, odd) head pair
            for st in range(NST):
                for p in range(4):
                    ones_ap = v_ext[:, st, p * 192 + 64 : p * 192 + 128]
                    if MMDT == F32R:
                        ones_ap = ones_ap.bitcast(F32)
                    nc.vector.memset(ones_ap, 1.0)

            # ---------------- Phase 1: projections ----------------
            with (
                tc.tile_pool(name="p1w", bufs=1) as p1w,
                tc.tile_pool(name="p1x", bufs=2) as p1x,
            ):
                w_q = p1w.tile([128, NKG, EH], MMDT)
                w_k = p1w.tile([128, NKG, EH], MMDT)
                w_v = p1w.tile([128, NKG, EH], MMDT)
                for kg in range(NKG):
                    sl = slice(kg * 128, (kg + 1) * 128)
                    nc.sync.dma_start(out=w_q[:, kg, :], in_=wqT_d[sl, :])
                    nc.sync.dma_start(out=w_k[:, kg, :], in_=wkT_d[sl, :])
                    nc.sync.dma_start(out=w_v[:, kg, :], in_=wvT_d[sl, :])

                for sb in range(NSB):
                    ssl = slice(sb * SB, (sb + 1) * SB)
                    xts = p1x.tile([128, NKG, SB], MMDT, tag="xts")
                    for kg in range(NKG):
                        nc.sync.dma_start(
                            out=xts[:, kg, :],
                            in_=xT_d[kg * 128 : (kg + 1) * 128, ssl],
                        )
                    # qT / kT blocks: out [e-part(128 of group mt), s(512)]
                    for w_sb, dst in ((w_q, qT), (w_k, kT)):
                        for mt in range(NEG):
                            ps = accp.tile([128, SB], F32, tag="acc")
                            msl = slice(mt * 128, (mt + 1) * 128)
                            for kg in range(NKG):
                                nc.tensor.matmul(
                                    out=ps,
                                    lhsT=(w_sb[:, kg, msl]),
                                    rhs=(xts[:, kg, :]),
                                    start=(kg == 0),
                                    stop=(kg == NKG - 1),
                                )
                            nc.vector.tensor_copy(dst[:, mt, ssl], ps)
                    # v blocks: out [s-part(128 of tile st), e(512)]
                    for st4 in range(SB // 128):
                        st = sb * (SB // 128) + st4
                        ps = accp.tile([128, EH], F32, tag="acc")
                        xsl = slice(st4 * 128, (st4 + 1) * 128)
                        for kg in range(NKG):
                            nc.tensor.matmul(
                                out=ps,
                                lhsT=(xts[:, kg, xsl]),
                                rhs=(w_v[:, kg, :]),
                                start=(kg == 0),
                                stop=(kg == NKG - 1),
                            )
                        # psum cols: head h at [h*64, h*64+64); dest pair p:
                        # even head -> p*192, odd head -> p*192+128
                        psr = ps[:].rearrange("p (a c) -> p a c", c=128)
                        vst = v_ext[:, st, :].rearrange("p (a w) -> p a w", w=192)
                        nc.vector.tensor_copy(vst[:, :, 0:64], psr[:, :, 0:64])
                        nc.vector.tensor_copy(vst[:, :, 128:192], psr[:, :, 64:128])

            # ---------------- Phase 2: attention ----------------
            # Heads processed in (even, odd) pairs sharing one e-group:
            # two concurrent K=64 score matmuls (row groups 0/64) fill the
            # whole PE array; one strided exp covers both heads' tiles.
            with (
                tc.tile_pool(name="p2c", bufs=1) as p2c,
                tc.tile_pool(name="ctxp", bufs=1) as ctxp,
                tc.tile_pool(name="expp", bufs=3) as expp,
                tc.tile_pool(name="sp", bufs=2, space="PSUM") as sp,
                tc.tile_pool(name="bcp", bufs=1, space="PSUM") as bcp,
                tc.tile_pool(name="smallp", bufs=2) as smallp,
            ):
                masks_sb = p2c.tile([128, 128], MMDT)
                nc.sync.dma_start(out=masks_sb, in_=masks_d[:, :])

                ctxT = ctxp.tile([128, NEG, S], MMDT)
                for pr in range(4):            # head pair: h=2pr (rows 0:64), h=2pr+1 (rows 64:128)
                    for ib in range(NSB):
                        isl = slice(ib * SB, (ib + 1) * SB)
                        njt = 4 * (ib + 1)
                        ps_c0 = accp.tile([128, SB], F32, tag="acc")
                        ps_c1 = accp.tile([128, SB], F32, tag="acc")
                        for jt in range(njt):
                            r = jt - 4 * ib
                            f0 = 128 * r if r > 0 else 0
                            jsl = slice(jt * 128, (jt + 1) * 128)
                            qsl = slice(ib * SB + f0, (ib + 1) * SB)
                            ps_s = sp.tile([128, 2 * SB], F32, tag="s")
                            nc.tensor.matmul(
                                out=ps_s[:, f0:SB],
                                lhsT=kT[0:64, pr, jsl],
                                rhs=qT[0:64, pr, qsl],
                                start=True,
                                stop=True,
                            )
                            nc.tensor.matmul(
                                out=ps_s[:, SB + f0 : 2 * SB],
                                lhsT=kT[64:128, pr, jsl],
                                rhs=qT[64:128, pr, qsl],
                                start=True,
                                stop=True,
                            )
                            expT = expp.tile([128, 2 * SB], MMDT, tag="exp")
                            ps_v = ps_s[:].rearrange("p (t c) -> p t c", t=2)
                            ex_v = expT[:].rearrange("p (t c) -> p t c", t=2)
                            nc.scalar.activation(
                                out=ex_v[:, :, f0:SB],
                                in_=ps_v[:, :, f0:SB],
                                func=mybir.ActivationFunctionType.Exp,
                                scale=1.0 / np.sqrt(HD),
                            )
                            if r >= 0:
                                nc.vector.tensor_mul(
                                    ex_v[:, :, f0 : f0 + 128],
                                    ex_v[:, :, f0 : f0 + 128],
                                    masks_sb[:].unsqueeze(1).broadcast_to(
                                        (128, 2, 128)
                                    ),
                                )
                            for t, ps_c in ((0, ps_c0), (1, ps_c1)):
                                coff = pr * 192 + 64 * t
                                nc.tensor.matmul(
                                    out=ps_c[:, f0:SB],
                                    lhsT=v_ext[:, jt, coff : coff + 128],
                                    rhs=expT[:, t * SB + f0 : (t + 1) * SB],
                                    start=(jt == 0),
                                    stop=(jt == njt - 1),
                                )
                        # normalize head 2pr (ctx rows 0:64, denom rows 64:128)
                        den = smallp.tile([128, SB], F32, tag="den")
                        nc.vector.tensor_copy(den[64:65, :], ps_c0[64:65, :])
                        bc_ps = bcp.tile([128, SB], F32, tag="bc")
                        nc.tensor.matmul(
                            out=bc_ps,
                            lhsT=ones_t[64:65, :],
                            rhs=den[64:65, :],
                            start=True,
                            stop=True,
                        )
                        braw = smallp.tile([128, SB], F32, tag="braw")
                        nc.vector.tensor_copy(braw[0:64, :], bc_ps[0:64, :])
                        rdt = smallp.tile([128, SB], F32, tag="rdt")
                        nc.vector.reciprocal_approx_fast(
                            rdt[0:64, :], braw[0:64, :]
                        )
                        nc.vector.tensor_mul(
                            ctxT[0:64, pr, isl], ps_c0[0:64, :], rdt[0:64, :]
                        )
                        # normalize head 2pr+1 (denom rows 0:64, ctx rows 64:128)
                        den1 = smallp.tile([128, SB], F32, tag="den1")
                        nc.vector.tensor_copy(den1[0:64, :], ps_c1[0:64, :])
                        rdt1 = smallp.tile([128, SB], F32, tag="rdt1")
                        nc.vector.reciprocal_approx_fast(
                            rdt1[0:64, :], den1[0:64, :]
                        )
                        bc_ps1 = bcp.tile([128, SB], F32, tag="bc")
                        nc.tensor.matmul(
                            out=bc_ps1,
                            lhsT=ones_t[0:1, :],
                            rhs=rdt1[0:1, :],
                            start=True,
                            stop=True,
                        )
                        bcs = smallp.tile([128, SB], F32, tag="bcs")
                        nc.vector.tensor_copy(bcs[64:128, :], bc_ps1[64:128, :])
                        nc.vector.tensor_mul(
                            ctxT[64:128, pr, isl], ps_c1[64:128, :], bcs[64:128, :]
                        )

                # ---------------- Phase 3: output projection ----------------
                with tc.tile_pool(name="p3", bufs=2) as p3:
                    woT_sb = p2c.tile([128, NEG, D], MMDT)
                    for gg in range(NEG):
                        nc.sync.dma_start(
                            out=woT_sb[:, gg, :],
                            in_=woT_d[gg * 128 : (gg + 1) * 128, :],
                        )
                    for it in range(NST):
                        itsl = slice(it * 128, (it + 1) * 128)
                        for ob in range(2):
                            osl = slice(ob * SB, (ob + 1) * SB)
                            ps = accp.tile([128, SB], F32, tag="acc")
                            for gg in range(NEG):
                                nc.tensor.matmul(
                                    out=ps,
                                    lhsT=(ctxT[:, gg, itsl]),
                                    rhs=(woT_sb[:, gg, osl]),
                                    start=(gg == 0),
                                    stop=(gg == NEG - 1),
                                )
                            ot = p3.tile([128, SB], F32, tag="ot")
                            nc.vector.tensor_copy(ot, ps)
                            nc.sync.dma_start(out=out_d[itsl, osl], in_=ot)

    nc.finalize()
    return nc


_NC = None


def _get_nc():
    global _NC
    if _NC is None:
        _NC = _build()
    return _NC


def kernel(x, wq, wk, wv, wo, wo_b):
    global LAST_RESULT
    x = np.ascontiguousarray(np.asarray(x, dtype=np.float32))
    wq = np.asarray(wq, dtype=np.float32)
    wk = np.asarray(wk, dtype=np.float32)
    wv = np.asarray(wv, dtype=np.float32)
    wo = np.asarray(wo, dtype=np.float32)
    wo_b = np.asarray(wo_b, dtype=np.float32)

    pp, ff = np.ogrid[0:128, 0:128]
    masks = (pp <= ff).astype(np.float32)

    in_maps = []
    for c in range(NCORES):
        b, hh = c // 2, c % 2
        es = slice(hh * EH, (hh + 1) * EH)
        in_maps.append(
            {
                "xt": np.ascontiguousarray(x[b].T.astype(MMNP)),
                "wqt": np.ascontiguousarray(wq[es, :].T.astype(MMNP)),
                "wkt": np.ascontiguousarray(wk[es, :].T.astype(MMNP)),
                "wvt": np.ascontiguousarray(wv[es, :].T.astype(MMNP)),
                "wot": np.ascontiguousarray(wo[:, es].T.astype(MMNP)),
                "masks": masks.astype(MMNP),
            }
        )

    nc = _get_nc()
    res = run_bass_kernel_spmd(nc, in_maps, list(range(NCORES)), trace=TRACE)
    LAST_RESULT = res

    out = np.empty((B, S, D), np.float32)
    for b in range(B):
        out[b] = res.results[2 * b]["out"] + res.results[2 * b + 1]["out"]
    out += wo_b[None, None, :]
    return out


# revision 17
# speedup vs baseline: 1.6742x; 1.2311x over previous
"""Multi-head causal attention on 8 Trainium2 NeuronCores.

Sharding: core c -> (batch b = c//2, head-half hh = c%2).  Each core computes
q/k/v projections for its 8 heads (column-sharded wq/wk/wv), causal attention,
and a full-width partial output projection (row-sharded wo).  Host sums the
two partials per batch and adds the bias.

Device-side layout trick: scores are computed transposed (scoresT[j, i]) so
that the softmax-weighted sum over keys (ctx) is a plain matmul with v as the
stationary operand.  Ones-columns baked alongside v produce the softmax
denominator replicated across 64 partitions in the same PSUM tile as ctx.
"""

import numpy as np

import concourse.bass as bass
import concourse.mybir as mybir
import concourse.tile as tile
from concourse import bacc
from concourse.bass_utils import run_bass_kernel_spmd

# Problem shape (hardcoded; kernel.py must be self-contained).
B, S, D, H = 4, 2048, 1024, 16
HD = D // H           # 64 head dim
NCORES = 8
EH = D // 2           # 512: per-core e-width (8 heads)
NHL = H // 2          # 8 local heads per core
SB = 512              # s-block (free dim of most matmuls)
NSB = S // SB         # 4
NST = S // 128        # 16 s-tiles / j-tiles
NEG = EH // 128       # 4 e-groups of 128 partitions
NKG = D // 128        # 8 d-groups (contraction tiles)
VROW = 4 * 192        # v_ext row: 4x [v_even(64) | ones(64) | v_odd(64)] = 768

F32 = mybir.dt.float32
F32R = mybir.dt.float32r
BF16 = mybir.dt.bfloat16
MMDT = F32R          # dtype for matmul inputs (BF16 or F32R)
import ml_dtypes
MMNP = ml_dtypes.bfloat16 if MMDT == BF16 else np.float32

TRACE = False
LAST_RESULT = None


def _build():
    nc = bacc.Bacc()

    xT_d = nc.dram_tensor("xt", [D, S], MMDT, kind="ExternalInput")
    wqT_d = nc.dram_tensor("wqt", [D, EH], MMDT, kind="ExternalInput")
    wkT_d = nc.dram_tensor("wkt", [D, EH], MMDT, kind="ExternalInput")
    wvT_d = nc.dram_tensor("wvt", [D, EH], MMDT, kind="ExternalInput")
    woT_d = nc.dram_tensor("wot", [EH, D], MMDT, kind="ExternalInput")
    masks_d = nc.dram_tensor("masks", [128, 128], MMDT, kind="ExternalInput")
    out_d = nc.dram_tensor("out", [S, D], F32, kind="ExternalOutput")
    scr_d = nc.dram_tensor("dscr", [4, NSB, 2, SB], F32)

    with tile.TileContext(nc) as tc:
        with (
            tc.tile_pool(name="persist", bufs=1) as persist,
            tc.tile_pool(name="accp", bufs=4, space="PSUM") as accp,
        ):
            qT = persist.tile([128, NEG, S], MMDT)      # [e-part, e-group, s]
            kT = persist.tile([128, NEG, S], MMDT)
            v_ext = persist.tile([128, NST, VROW], MMDT)  # [s-part, s-tile, row]

            # shared ones block between each (even, odd) head pair
            for st in range(NST):
                for p in range(4):
                    ones_ap = v_ext[:, st, p * 192 + 64 : p * 192 + 128]
                    if MMDT == F32R:
                        ones_ap = ones_ap.bitcast(F32)
                    nc.vector.memset(ones_ap, 1.0)

            # ---------------- Phase 1: projections ----------------
            with (
                tc.tile_pool(name="p1w", bufs=1) as p1w,
                tc.tile_pool(name="p1x", bufs=2) as p1x,
            ):
                w_q = p1w.tile([128, NKG, EH], MMDT)
                w_k = p1w.tile([128, NKG, EH], MMDT)
                w_v = p1w.tile([128, NKG, EH], MMDT)
                for kg in range(NKG):
                    sl = slice(kg * 128, (kg + 1) * 128)
                    nc.sync.dma_start(out=w_q[:, kg, :], in_=wqT_d[sl, :])
                    nc.sync.dma_start(out=w_k[:, kg, :], in_=wkT_d[sl, :])
                    nc.sync.dma_start(out=w_v[:, kg, :], in_=wvT_d[sl, :])

                for sb in range(NSB):
                    ssl = slice(sb * SB, (sb + 1) * SB)
                    xts = p1x.tile([128, NKG, SB], MMDT, tag="xts")
                    for kg in range(NKG):
                        nc.sync.dma_start(
                            out=xts[:, kg, :],
                            in_=xT_d[kg * 128 : (kg + 1) * 128, ssl],
                        )
                    # qT / kT blocks: out [e-part(128 of group mt), s(512)]
                    for w_sb, dst in ((w_q, qT), (w_k, kT)):
                        for mt in range(NEG):
                            ps = accp.tile([128, SB], F32, tag="acc")
                            msl = slice(mt * 128, (mt + 1) * 128)
                            for kg in range(NKG):
                                nc.tensor.matmul(
                                    out=ps,
                                    lhsT=(w_sb[:, kg, msl]),
                                    rhs=(xts[:, kg, :]),
                                    start=(kg == 0),
                                    stop=(kg == NKG - 1),
                                )
                            nc.vector.tensor_copy(dst[:, mt, ssl], ps)
                    # v blocks: out [s-part(128 of tile st), e(512)]
                    for st4 in range(SB // 128):
                        st = sb * (SB // 128) + st4
                        ps = accp.tile([128, EH], F32, tag="acc")
                        xsl = slice(st4 * 128, (st4 + 1) * 128)
                        for kg in range(NKG):
                            nc.tensor.matmul(
                                out=ps,
                                lhsT=(xts[:, kg, xsl]),
                                rhs=(w_v[:, kg, :]),
                                start=(kg == 0),
                                stop=(kg == NKG - 1),
                            )
                        # psum cols: head h at [h*64, h*64+64); dest pair p:
                        # even head -> p*192, odd head -> p*192+128
                        psr = ps[:].rearrange("p (a c) -> p a c", c=128)
                        vst = v_ext[:, st, :].rearrange("p (a w) -> p a w", w=192)
                        nc.vector.tensor_copy(vst[:, :, 128:192], psr[:, :, 0:64])
                        nc.vector.tensor_copy(vst[:, :, 0:64], psr[:, :, 64:128])

            # ---------------- Phase 2: attention ----------------
            # Heads processed in (even, odd) pairs sharing one e-group:
            # two concurrent K=64 score matmuls (row groups 0/64) fill the
            # whole PE array; one strided exp covers both heads' tiles.
            with (
                tc.tile_pool(name="p2c", bufs=1) as p2c,
                tc.tile_pool(name="ctxp", bufs=1) as ctxp,
                tc.tile_pool(name="expp", bufs=3) as expp,
                tc.tile_pool(name="sp", bufs=2, space="PSUM") as sp,
                tc.tile_pool(name="smallp", bufs=2) as smallp,
            ):
                masks_sb = p2c.tile([128, 128], MMDT)
                nc.sync.dma_start(out=masks_sb, in_=masks_d[:, :])

                ctxT = ctxp.tile([128, NEG, S], MMDT)
                for pr in range(4):            # head pair: h=2pr (rows 0:64), h=2pr+1 (rows 64:128)
                    for ib in range(NSB):
                        isl = slice(ib * SB, (ib + 1) * SB)
                        njt = 4 * (ib + 1)
                        ps_c0 = accp.tile([128, SB], F32, tag="acc")
                        ps_c1 = accp.tile([128, SB], F32, tag="acc")
                        for jt in range(njt):
                            r = jt - 4 * ib
                            f0 = 128 * r if r > 0 else 0
                            jsl = slice(jt * 128, (jt + 1) * 128)
                            qsl = slice(ib * SB + f0, (ib + 1) * SB)
                            ps_s = sp.tile([128, 2 * SB], F32, tag="s")
                            nc.tensor.matmul(
                                out=ps_s[:, f0:SB],
                                lhsT=kT[0:64, pr, jsl],
                                rhs=qT[0:64, pr, qsl],
                                start=True,
                                stop=True,
                            )
                            nc.tensor.matmul(
                                out=ps_s[:, SB + f0 : 2 * SB],
                                lhsT=kT[64:128, pr, jsl],
                                rhs=qT[64:128, pr, qsl],
                                start=True,
                                stop=True,
                            )
                            expT = expp.tile([128, 2 * SB], MMDT, tag="exp")
                            ps_v = ps_s[:].rearrange("p (t c) -> p t c", t=2)
                            ex_v = expT[:].rearrange("p (t c) -> p t c", t=2)
                            nc.scalar.activation(
                                out=ex_v[:, :, f0:SB],
                                in_=ps_v[:, :, f0:SB],
                                func=mybir.ActivationFunctionType.Exp,
                                scale=1.0 / np.sqrt(HD),
                            )
                            if r >= 0:
                                nc.vector.tensor_mul(
                                    ex_v[:, :, f0 : f0 + 128],
                                    ex_v[:, :, f0 : f0 + 128],
                                    masks_sb[:].unsqueeze(1).broadcast_to(
                                        (128, 2, 128)
                                    ),
                                )
                            for t, ps_c in ((0, ps_c0), (1, ps_c1)):
                                coff = pr * 192 + (64 if t == 0 else 0)
                                nc.tensor.matmul(
                                    out=ps_c[:, f0:SB],
                                    lhsT=v_ext[:, jt, coff : coff + 128],
                                    rhs=expT[:, t * SB + f0 : (t + 1) * SB],
                                    start=(jt == 0),
                                    stop=(jt == njt - 1),
                                )
                        # even head (ps_c0): denom rows 0:64, ctx rows 64:128
                        den0 = smallp.tile([128, SB], F32, tag="den0")
                        nc.vector.tensor_copy(den0[0:64, :], ps_c0[0:64, :])
                        rdt0 = smallp.tile([128, SB], F32, tag="rdt0")
                        nc.vector.reciprocal_approx_fast(
                            rdt0[0:64, :], den0[0:64, :]
                        )
                        nc.sync.dma_start(
                            out=scr_d[pr, ib, 0, :], in_=rdt0[0:1, :]
                        )
                        se = scr_d[pr, ib, 0, :]
                        bce = smallp.tile([128, SB], F32, tag="bce")
                        nc.sync.dma_start(
                            out=bce[64:128, :],
                            in_=bass.AP(
                                tensor=se.tensor, offset=se.offset,
                                ap=[[0, 64], [1, SB]],
                            ),
                        )
                        nc.vector.tensor_mul(
                            ctxT[64:128, pr, isl], ps_c0[64:128, :], bce[64:128, :]
                        )
                        # odd head (ps_c1): ctx rows 0:64, denom rows 64:128
                        den1 = smallp.tile([128, SB], F32, tag="den1")
                        nc.vector.tensor_copy(den1[64:65, :], ps_c1[64:65, :])
                        nc.sync.dma_start(
                            out=scr_d[pr, ib, 1, :], in_=den1[64:65, :]
                        )
                        so = scr_d[pr, ib, 1, :]
                        braw = smallp.tile([128, SB], F32, tag="braw")
                        nc.sync.dma_start(
                            out=braw[0:64, :],
                            in_=bass.AP(
                                tensor=so.tensor, offset=so.offset,
                                ap=[[0, 64], [1, SB]],
                            ),
                        )
                        rdt1 = smallp.tile([128, SB], F32, tag="rdt1")
                        nc.vector.reciprocal_approx_fast(
                            rdt1[0:64, :], braw[0:64, :]
                        )
                        nc.vector.tensor_mul(
                            ctxT[0:64, pr, isl], ps_c1[0:64, :], rdt1[0:64, :]
                        )

            # ---------------- Phase 3: output projection ----------------
                with tc.tile_pool(name="p3", bufs=2) as p3:
                    woT_sb = p2c.tile([128, NEG, D], MMDT)
                    for gg in range(NEG):
                        nc.sync.dma_start(
                            out=woT_sb[:, gg, :],
                            in_=woT_d[gg * 128 : (gg + 1) * 128, :],
                        )
                    for it in range(NST):
                        itsl = slice(it * 128, (it + 1) * 128)
                        for ob in range(2):
                            osl = slice(ob * SB, (ob + 1) * SB)
                            ps = accp.tile([128, SB], F32, tag="acc")
                            for gg in range(NEG):
                                nc.tensor.matmul(
                                    out=ps,
                                    lhsT=(ctxT[:, gg, itsl]),
                                    rhs=(woT_sb[:, gg, osl]),
                                    start=(gg == 0),
                                    stop=(gg == NEG - 1),
                                )
                            ot = p3.tile([128, SB], F32, tag="ot")
                            nc.vector.tensor_copy(ot, ps)
                            nc.sync.dma_start(out=out_d[itsl, osl], in_=ot)

    nc.finalize()
    return nc


_NC = None


def _get_nc():
    global _NC
    if _NC is None:
        _NC = _build()
    return _NC


def kernel(x, wq, wk, wv, wo, wo_b):
    global LAST_RESULT
    x = np.ascontiguousarray(np.asarray(x, dtype=np.float32))
    wq = np.asarray(wq, dtype=np.float32)
    wk = np.asarray(wk, dtype=np.float32)
    wv = np.asarray(wv, dtype=np.float32)
    wo = np.asarray(wo, dtype=np.float32)
    wo_b = np.asarray(wo_b, dtype=np.float32)

    pp, ff = np.ogrid[0:128, 0:128]
    masks = (pp <= ff).astype(np.float32)

    in_maps = []
    for c in range(NCORES):
        b, hh = c // 2, c % 2
        es = slice(hh * EH, (hh + 1) * EH)
        in_maps.append(
            {
                "xt": np.ascontiguousarray(x[b].T.astype(MMNP)),
                "wqt": np.ascontiguousarray(wq[es, :].T.astype(MMNP)),
                "wkt": np.ascontiguousarray(wk[es, :].T.astype(MMNP)),
                "wvt": np.ascontiguousarray(wv[es, :].T.astype(MMNP)),
                "wot": np.ascontiguousarray(
                    wo[:, es].T.astype(MMNP)
                    .reshape(4, 2, 64, D)[:, ::-1]
                    .reshape(EH, D)
                ),
                "masks": masks.astype(MMNP),
            }
        )

    nc = _get_nc()
    res = run_bass_kernel_spmd(nc, in_maps, list(range(NCORES)), trace=TRACE)
    LAST_RESULT = res

    out = np.empty((B, S, D), np.float32)
    for b in range(B):
        out[b] = res.results[2 * b]["out"] + res.results[2 * b + 1]["out"]
    out += wo_b[None, None, :]
    return out


# revision 18
# speedup vs baseline: 1.7458x; 1.0427x over previous
"""Multi-head causal attention on 8 Trainium2 NeuronCores.

Sharding: core c -> (batch b = c//2, head-half hh = c%2).  Each core computes
q/k/v projections for its 8 heads (column-sharded wq/wk/wv), causal attention,
and a full-width partial output projection (row-sharded wo).  Host sums the
two partials per batch and adds the bias.

Device-side layout trick: scores are computed transposed (scoresT[j, i]) so
that the softmax-weighted sum over keys (ctx) is a plain matmul with v as the
stationary operand.  Ones-columns baked alongside v produce the softmax
denominator replicated across 64 partitions in the same PSUM tile as ctx.
"""

import numpy as np

import concourse.bass as bass
import concourse.mybir as mybir
import concourse.tile as tile
from concourse import bacc
from concourse.bass_utils import run_bass_kernel_spmd

# Problem shape (hardcoded; kernel.py must be self-contained).
B, S, D, H = 4, 2048, 1024, 16
HD = D // H           # 64 head dim
NCORES = 8
EH = D // 2           # 512: per-core e-width (8 heads)
NHL = H // 2          # 8 local heads per core
SB = 512              # s-block (free dim of most matmuls)
NSB = S // SB         # 4
NST = S // 128        # 16 s-tiles / j-tiles
NEG = EH // 128       # 4 e-groups of 128 partitions
NKG = D // 128        # 8 d-groups (contraction tiles)
VROW = 4 * 192        # v_ext row: 4x [v_even(64) | ones(64) | v_odd(64)] = 768

F32 = mybir.dt.float32
F32R = mybir.dt.float32r
BF16 = mybir.dt.bfloat16
MMDT = F32R          # dtype for matmul inputs (BF16 or F32R)
import ml_dtypes
MMNP = ml_dtypes.bfloat16 if MMDT == BF16 else np.float32

TRACE = False
LAST_RESULT = None


def _build():
    nc = bacc.Bacc()

    xT_d = nc.dram_tensor("xt", [D, S], MMDT, kind="ExternalInput")
    wqT_d = nc.dram_tensor("wqt", [D, EH], MMDT, kind="ExternalInput")
    wkT_d = nc.dram_tensor("wkt", [D, EH], MMDT, kind="ExternalInput")
    wvT_d = nc.dram_tensor("wvt", [D, EH], MMDT, kind="ExternalInput")
    woT_d = nc.dram_tensor("wot", [EH, D], MMDT, kind="ExternalInput")
    masks_d = nc.dram_tensor("masks", [128, 128], MMDT, kind="ExternalInput")
    out_d = nc.dram_tensor("out", [S, D], F32, kind="ExternalOutput")
    scr_d = nc.dram_tensor("dscr", [4, NSB, 2, SB], F32)

    with tile.TileContext(nc) as tc:
        with (
            tc.tile_pool(name="persist", bufs=1) as persist,
            tc.tile_pool(name="accp", bufs=4, space="PSUM") as accp,
        ):
            qT = persist.tile([128, NEG, S], MMDT)      # [e-part, e-group, s]
            kT = persist.tile([128, NEG, S], MMDT)
            v_ext = persist.tile([128, NST, VROW], MMDT)  # [s-part, s-tile, row]

            # shared ones block between each (even, odd) head pair
            for st in range(NST):
                for p in range(4):
                    ones_ap = v_ext[:, st, p * 192 + 64 : p * 192 + 128]
                    if MMDT == F32R:
                        ones_ap = ones_ap.bitcast(F32)
                    nc.vector.memset(ones_ap, 1.0)

            # ---------------- Phase 1: projections ----------------
            with (
                tc.tile_pool(name="p1w", bufs=1) as p1w,
                tc.tile_pool(name="p1x", bufs=2) as p1x,
            ):
                w_q = p1w.tile([128, NKG, EH], MMDT)
                w_k = p1w.tile([128, NKG, EH], MMDT)
                w_v = p1w.tile([128, NKG, EH], MMDT)
                for kg in range(NKG):
                    sl = slice(kg * 128, (kg + 1) * 128)
                    nc.sync.dma_start(out=w_q[:, kg, :], in_=wqT_d[sl, :])
                    nc.sync.dma_start(out=w_k[:, kg, :], in_=wkT_d[sl, :])
                    nc.sync.dma_start(out=w_v[:, kg, :], in_=wvT_d[sl, :])

                for sb in range(NSB):
                    ssl = slice(sb * SB, (sb + 1) * SB)
                    xts = p1x.tile([128, NKG, SB], MMDT, tag="xts")
                    for kg in range(NKG):
                        nc.sync.dma_start(
                            out=xts[:, kg, :],
                            in_=xT_d[kg * 128 : (kg + 1) * 128, ssl],
                        )
                    # qT / kT blocks: out [e-part(128 of group mt), s(512)]
                    for w_sb, dst in ((w_q, qT), (w_k, kT)):
                        for mt in range(NEG):
                            ps = accp.tile([128, SB], F32, tag="acc")
                            msl = slice(mt * 128, (mt + 1) * 128)
                            for kg in range(NKG):
                                nc.tensor.matmul(
                                    out=ps,
                                    lhsT=(w_sb[:, kg, msl]),
                                    rhs=(xts[:, kg, :]),
                                    start=(kg == 0),
                                    stop=(kg == NKG - 1),
                                )
                            nc.vector.tensor_copy(dst[:, mt, ssl], ps)
                    # v blocks: out [s-part(128 of tile st), e(512)]
                    for st4 in range(SB // 128):
                        st = sb * (SB // 128) + st4
                        ps = accp.tile([128, EH], F32, tag="acc")
                        xsl = slice(st4 * 128, (st4 + 1) * 128)
                        for kg in range(NKG):
                            nc.tensor.matmul(
                                out=ps,
                                lhsT=(xts[:, kg, xsl]),
                                rhs=(w_v[:, kg, :]),
                                start=(kg == 0),
                                stop=(kg == NKG - 1),
                            )
                        # psum cols: head h at [h*64, h*64+64); dest pair p:
                        # even head -> p*192, odd head -> p*192+128
                        psr = ps[:].rearrange("p (a c) -> p a c", c=128)
                        vst = v_ext[:, st, :].rearrange("p (a w) -> p a w", w=192)
                        nc.vector.tensor_copy(vst[:, :, 128:192], psr[:, :, 0:64])
                        nc.vector.tensor_copy(vst[:, :, 0:64], psr[:, :, 64:128])

            # ---------------- Phase 2: attention ----------------
            # Heads processed in (even, odd) pairs sharing one e-group:
            # two concurrent K=64 score matmuls (row groups 0/64) fill the
            # whole PE array; one strided exp covers both heads' tiles.
            with (
                tc.tile_pool(name="p2c", bufs=1) as p2c,
                tc.tile_pool(name="ctxp", bufs=1) as ctxp,
                tc.tile_pool(name="expp", bufs=4) as expp,
                tc.tile_pool(name="sp", bufs=2, space="PSUM") as sp,
                tc.tile_pool(name="smallp", bufs=2) as smallp,
            ):
                masks_sb = p2c.tile([128, 128], MMDT)
                nc.sync.dma_start(out=masks_sb, in_=masks_d[:, :])

                ctxT = ctxp.tile([128, NEG, S], MMDT)
                for pr in range(4):            # head pair: h=2pr (rows 0:64), h=2pr+1 (rows 64:128)
                    for ib in range(NSB):
                        isl = slice(ib * SB, (ib + 1) * SB)
                        njt = 4 * (ib + 1)
                        ps_c0 = accp.tile([128, SB], F32, tag="acc")
                        ps_c1 = accp.tile([128, SB], F32, tag="acc")

                        def scores(jt):
                            r = jt - 4 * ib
                            f0 = 128 * r if r > 0 else 0
                            jsl = slice(jt * 128, (jt + 1) * 128)
                            qsl = slice(ib * SB + f0, (ib + 1) * SB)
                            ps_s = sp.tile([128, 2 * SB], F32, tag="s")
                            nc.tensor.matmul(
                                out=ps_s[:, f0:SB],
                                lhsT=kT[0:64, pr, jsl],
                                rhs=qT[0:64, pr, qsl],
                                start=True,
                                stop=True,
                            )
                            nc.tensor.matmul(
                                out=ps_s[:, SB + f0 : 2 * SB],
                                lhsT=kT[64:128, pr, jsl],
                                rhs=qT[64:128, pr, qsl],
                                start=True,
                                stop=True,
                            )
                            return ps_s

                        def softmax_ctx(jt, ps_s):
                            r = jt - 4 * ib
                            f0 = 128 * r if r > 0 else 0
                            expT = expp.tile([128, 2 * SB], MMDT, tag="exp")
                            ps_v = ps_s[:].rearrange("p (t c) -> p t c", t=2)
                            ex_v = expT[:].rearrange("p (t c) -> p t c", t=2)
                            nc.scalar.activation(
                                out=ex_v[:, :, f0:SB],
                                in_=ps_v[:, :, f0:SB],
                                func=mybir.ActivationFunctionType.Exp,
                                scale=1.0 / np.sqrt(HD),
                            )
                            if r >= 0:
                                nc.vector.tensor_mul(
                                    ex_v[:, :, f0 : f0 + 128],
                                    ex_v[:, :, f0 : f0 + 128],
                                    masks_sb[:].unsqueeze(1).broadcast_to(
                                        (128, 2, 128)
                                    ),
                                )
                            for t, ps_c in ((0, ps_c0), (1, ps_c1)):
                                coff = pr * 192 + (64 if t == 0 else 0)
                                nc.tensor.matmul(
                                    out=ps_c[:, f0:SB],
                                    lhsT=v_ext[:, jt, coff : coff + 128],
                                    rhs=expT[:, t * SB + f0 : (t + 1) * SB],
                                    start=(jt == 0),
                                    stop=(jt == njt - 1),
                                )

                        # software pipeline: scores run one jt ahead of
                        # exp/ctx so the in-order PE queue never blocks
                        prev = None
                        for jt in range(njt):
                            ps_prev = prev
                            prev = (jt, scores(jt))
                            if ps_prev is not None:
                                softmax_ctx(*ps_prev)
                        softmax_ctx(*prev)
                        # even head (ps_c0): denom rows 0:64, ctx rows 64:128
                        den0 = smallp.tile([128, SB], F32, tag="den0")
                        nc.vector.tensor_copy(den0[0:64, :], ps_c0[0:64, :])
                        rdt0 = smallp.tile([128, SB], F32, tag="rdt0")
                        nc.vector.reciprocal_approx_fast(
                            rdt0[0:64, :], den0[0:64, :]
                        )
                        nc.sync.dma_start(
                            out=scr_d[pr, ib, 0, :], in_=rdt0[0:1, :]
                        )
                        se = scr_d[pr, ib, 0, :]
                        bce = smallp.tile([128, SB], F32, tag="bce")
                        nc.sync.dma_start(
                            out=bce[64:128, :],
                            in_=bass.AP(
                                tensor=se.tensor, offset=se.offset,
                                ap=[[0, 64], [1, SB]],
                            ),
                        )
                        nc.vector.tensor_mul(
                            ctxT[64:128, pr, isl], ps_c0[64:128, :], bce[64:128, :]
                        )
                        # odd head (ps_c1): ctx rows 0:64, denom rows 64:128
                        den1 = smallp.tile([128, SB], F32, tag="den1")
                        nc.vector.tensor_copy(den1[64:65, :], ps_c1[64:65, :])
                        nc.sync.dma_start(
                            out=scr_d[pr, ib, 1, :], in_=den1[64:65, :]
                        )
                        so = scr_d[pr, ib, 1, :]
                        braw = smallp.tile([128, SB], F32, tag="braw")
                        nc.sync.dma_start(
                            out=braw[0:64, :],
                            in_=bass.AP(
                                tensor=so.tensor, offset=so.offset,
                                ap=[[0, 64], [1, SB]],
                            ),
                        )
                        rdt1 = smallp.tile([128, SB], F32, tag="rdt1")
                        nc.vector.reciprocal_approx_fast(
                            rdt1[0:64, :], braw[0:64, :]
                        )
                        nc.vector.tensor_mul(
                            ctxT[0:64, pr, isl], ps_c1[0:64, :], rdt1[0:64, :]
                        )

            # ---------------- Phase 3: output projection ----------------
                with tc.tile_pool(name="p3", bufs=2) as p3:
                    woT_sb = p2c.tile([128, NEG, D], MMDT)
                    for gg in range(NEG):
                        nc.sync.dma_start(
                            out=woT_sb[:, gg, :],
                            in_=woT_d[gg * 128 : (gg + 1) * 128, :],
                        )
                    for it in range(NST):
                        itsl = slice(it * 128, (it + 1) * 128)
                        for ob in range(2):
                            osl = slice(ob * SB, (ob + 1) * SB)
                            ps = accp.tile([128, SB], F32, tag="acc")
                            for gg in range(NEG):
                                nc.tensor.matmul(
                                    out=ps,
                                    lhsT=(ctxT[:, gg, itsl]),
                                    rhs=(woT_sb[:, gg, osl]),
                                    start=(gg == 0),
                                    stop=(gg == NEG - 1),
                                )
                            ot = p3.tile([128, SB], F32, tag="ot")
                            nc.vector.tensor_copy(ot, ps)
                            nc.sync.dma_start(out=out_d[itsl, osl], in_=ot)

    nc.finalize()
    return nc


_NC = None


def _get_nc():
    global _NC
    if _NC is None:
        _NC = _build()
    return _NC


def kernel(x, wq, wk, wv, wo, wo_b):
    global LAST_RESULT
    x = np.ascontiguousarray(np.asarray(x, dtype=np.float32))
    wq = np.asarray(wq, dtype=np.float32)
    wk = np.asarray(wk, dtype=np.float32)
    wv = np.asarray(wv, dtype=np.float32)
    wo = np.asarray(wo, dtype=np.float32)
    wo_b = np.asarray(wo_b, dtype=np.float32)

    pp, ff = np.ogrid[0:128, 0:128]
    masks = (pp <= ff).astype(np.float32)

    in_maps = []
    for c in range(NCORES):
        b, hh = c // 2, c % 2
        es = slice(hh * EH, (hh + 1) * EH)
        in_maps.append(
            {
                "xt": np.ascontiguousarray(x[b].T.astype(MMNP)),
                "wqt": np.ascontiguousarray(wq[es, :].T.astype(MMNP)),
                "wkt": np.ascontiguousarray(wk[es, :].T.astype(MMNP)),
                "wvt": np.ascontiguousarray(wv[es, :].T.astype(MMNP)),
                "wot": np.ascontiguousarray(
                    wo[:, es].T.astype(MMNP)
                    .reshape(4, 2, 64, D)[:, ::-1]
                    .reshape(EH, D)
                ),
                "masks": masks.astype(MMNP),
            }
        )

    nc = _get_nc()
    res = run_bass_kernel_spmd(nc, in_maps, list(range(NCORES)), trace=TRACE)
    LAST_RESULT = res

    out = np.empty((B, S, D), np.float32)
    for b in range(B):
        out[b] = res.results[2 * b]["out"] + res.results[2 * b + 1]["out"]
    out += wo_b[None, None, :]
    return out
